# revision 14
# baseline (speedup 1.0000x reference)
"""Trainium2 Bass kernel for nn_ErdosLoss (graph loss function).

Math (reference reformulated, validated to ~1e-6 rel err):
  penalty:  log_score = scatter_add(log(1 - p + 1e-6), tgt)   over N nodes
            loss2 = mean(exp(log_score)) * 9600
  loss3:    p @ triu(H H^T, 1) @ p^T  ==  (||s||^2 - sum_e d_e p_e^2) / 2
            where s = scatter_add(p, tgt) + scatter_add(p, src | src != tgt),
            d_e = 2 - m_e, m_e = (src_e == tgt_e).
  out = loss2 + 200 * loss3 / num_graphs,  num_graphs = max(batch) + 1.

Single-launch single-core design (per-NEFF fixed overhead here is ~11us, so
any second launch loses):
  Host (index-only preprocessing; values are only reordered, never
  combined): sort the (node, value) scatter pairs by node, bin them so that
  partition p holds exactly nodes [32p, 32(p+1)), and emit aligned [128, K]
  arrays per list: V (values), RM (0 at each node's first pair, else 1),
  LM (1 at each node's last pair, else 0).

  Device: Ln on ACT; segment-local running sum on DVE tensor_tensor_scan
  (state = RM*state + v, resetting at segment starts); d = r*LM isolates
  node totals; exp(d) row-accumulated counts masked slots as exp(0)=1 -> a
  compile-time correction (128*K1 - 4000; empty nodes cancel).  s-list:
  same scan; dsq = rowsum(d2^2) - rowsum(V2^2) with the V2^2 half on ACT
  (Square) and the [128,1] diff on Pool, keeping DVE's serial chain short.
  R = [SC*er - SC*corr/128 | dsq]; ones-matmul partition-reduce to PSUM
  [1,2]; res = F1*(100/ng) + F0; one [1,1] DMA out.

  Latency layout: one explicit ACT table load (set 6 = Ln+Exp+Copy) so no
  mid-kernel reloads; inputs on three independent queues sized by
  need-time: V1 (f32, longest chain) via the gpsimd SWDGE queue (earliest
  transfer start), the f16 log-masks via sync, the f16 s-list via scalar.
  Masks and s-values ride as f16 (exact for 0/1; s-path tolerance ~1e-3 vs
  the 2e-2 gate).
"""

import numpy as np

import concourse.bacc as bacc
import concourse.mybir as mybir
import concourse.tile as tile
from concourse import bass_utils

F32 = mybir.dt.float32
F16 = mybir.dt.float16
ALU = mybir.AluOpType
ACT = mybir.ActivationFunctionType

N_NODES = 4000
PENALTY_SCALE = 16 * 200 * 3   # 9600
SC = PENALTY_SCALE / N_NODES   # 2.4
NPP = 32                       # nodes per partition (128 * 32 = 4096 >= 4000)

K1 = 64    # log-list slots per partition  (measured max 63)
K2 = 124   # s-list slots per partition    (measured max 120)


def _build(k1: int, k2: int):
    nc = bacc.Bacc("TRN2", target_bir_lowering=False, debug=False, num_devices=1)

    da = nc.dram_tensor("dina", [128, k1 + 1], F32, kind="ExternalInput").ap()
    dm = nc.dram_tensor("dinm", [128, 2 * k1], F16, kind="ExternalInput").ap()
    db = nc.dram_tensor("dinb", [128, 3 * k2], F16, kind="ExternalInput").ap()
    outd = nc.dram_tensor("out", [1, 1], F32, kind="ExternalOutput").ap()

    with tile.TileContext(nc) as tc:
        with (
            tc.tile_pool(name="pool", bufs=1) as pool,
            tc.tile_pool(name="psum", bufs=1, space="PSUM") as ppool,
        ):
            # one ACT table set covering Ln+Exp+Copy+Square (set 6); the
            # auto placement pass then inserts no further loads
            nc.scalar.add_instruction(mybir.InstLoadActFuncSet(
                name="actload6", ins=[], outs=[], act_func_set_id=6))

            # inputs on the two HWDGE queues: V1 on sync, masks on scalar
            # (both land ~equally early), then the s-list split by partition
            # halves across both queues
            Ba = pool.tile([128, k1 + 1], F32, tag="Ba")
            nc.sync.dma_start(Ba[:], da)
            Bm = pool.tile([128, 2 * k1], F16, tag="Bm")
            nc.scalar.dma_start(Bm[:], dm)
            Bb = pool.tile([128, 3 * k2], F16, tag="Bb")
            nc.sync.dma_start(Bb[0:64, :], db[0:64, :])
            nc.scalar.dma_start(Bb[64:128, :], db[64:128, :])

            # constants
            wb = pool.tile([128, 1], F32, tag="wb")
            nc.vector.memset(wb[:], 0.0)
            bias1 = pool.tile([128, 1], F32, tag="bias1")
            nc.vector.memset(bias1[:], 1.0 + 1e-6)
            ones_t = pool.tile([128, 1], F32, tag="ones_t")
            nc.vector.memset(ones_t[:], 1.0)
            # warm the DVE scan path on dummy data
            ws = pool.tile([128, 4], F32, tag="ws")
            nc.vector.tensor_tensor_scan(
                ws[:], wb[:].to_broadcast((128, 4)), wb[:].to_broadcast((128, 4)),
                0.0, op0=ALU.mult, op1=ALU.add,
            )

            V1 = Ba[:, 0:k1]
            bmax = Ba[0:1, k1:k1 + 1]
            RM1 = Bm[:, 0:k1]
            LM1 = Bm[:, k1:2 * k1]
            V2 = Bb[:, 0:k2]
            RM2 = Bb[:, k2:2 * k2]
            LM2 = Bb[:, 2 * k2:3 * k2]

            R = pool.tile([128, 2], F32, tag="R")

            # num_graphs early (only needs Ba): rng = 100 / (max(batch) + 1)
            ng1 = pool.tile([1, 1], F32, tag="ng1")
            nc.vector.tensor_scalar(ng1[:], bmax, 1.0, 0.01,
                                    op0=ALU.add, op1=ALU.mult)
            rng = pool.tile([1, 1], F32, tag="rng")
            nc.vector.reciprocal(rng[:], ng1[:])

            # ---- log path (critical): Ln -> scan -> mask -> Exp+accum
            Lv = pool.tile([128, k1], F32, tag="Lv")
            nc.scalar.activation(Lv[:], V1, ACT.Ln, scale=-1.0, bias=bias1[:])
            r1 = pool.tile([128, k1], F32, tag="r1")
            nc.vector.tensor_tensor_scan(
                r1[:], RM1, Lv[:], 0.0, op0=ALU.mult, op1=ALU.add
            )
            dm1 = pool.tile([128, k1], F32, tag="dm1")
            nc.vector.tensor_tensor(dm1[:], r1[:], LM1, op=ALU.mult)
            e1 = pool.tile([128, k1], F32, tag="e1")
            er = pool.tile([128, 1], F32, tag="er")
            nc.scalar.activation(e1[:], dm1[:], ACT.Exp, bias=wb[:],
                                 accum_out=er[:])

            # ---- s path on DVE: scan -> mask -> sum (d2^2 - V2^2) as
            # (d2 - V2)(d2 + V2) accumulated in one pass
            r2 = pool.tile([128, k2], F32, tag="r2")
            nc.vector.tensor_tensor_scan(
                r2[:], RM2, V2, 0.0, op0=ALU.mult, op1=ALU.add
            )
            dm2 = pool.tile([128, k2], F32, tag="dm2")
            nc.vector.tensor_tensor(dm2[:], r2[:], LM2, op=ALU.mult)
            aa = pool.tile([128, k2], F32, tag="aa")
            nc.vector.tensor_tensor(aa[:], dm2[:], V2, op=ALU.subtract)
            bb = pool.tile([128, k2], F32, tag="bb")
            nc.vector.tensor_tensor(bb[:], dm2[:], V2, op=ALU.add)
            sq2 = pool.tile([128, k2], F32, tag="sq2")
            nc.vector.scalar_tensor_tensor(
                sq2[:], aa[:], 1.0, bb[:],
                op0=ALU.mult, op1=ALU.mult, accum_out=R[:, 1:2],
            )
            corr = float(128 * k1 - N_NODES)
            nc.vector.tensor_scalar(R[:, 0:1], er[:], SC, -corr * SC / 128.0,
                                    op0=ALU.mult, op1=ALU.add)

            # ---- partition reduce + final tail (DVE only)
            F = ppool.tile([1, 2], F32, tag="F")
            nc.tensor.matmul(F[:], ones_t[:], R[:], start=True, stop=True)
            cp0 = pool.tile([1, 1], F32, tag="cp0")
            nc.vector.tensor_copy(cp0[:], F[:, 0:1])
            res2 = pool.tile([1, 1], F32, tag="res2")
            nc.vector.scalar_tensor_tensor(
                res2[:], F[:, 1:2], rng[:], cp0[:],
                op0=ALU.mult, op1=ALU.add,
            )
            nc.sync.dma_start(outd, res2[:])

    nc.compile()
    return nc


def _pack_list(nodes, vals, K):
    """Sort (node, value) pairs, bin node n into partition n // 32, emit
    aligned V / RM / LM [128, K] arrays.  Index work + reordering only."""
    order = np.argsort(nodes, kind="stable")
    nodes = nodes[order]
    vals = vals[order]
    blk = nodes // NPP
    starts = np.searchsorted(blk, np.arange(128), "left")
    cnt = np.bincount(blk, minlength=128)
    if cnt.max() > K:
        return None
    pos = np.arange(len(nodes)) - starts[blk]

    V = np.zeros((128, K), np.float32)
    RM = np.ones((128, K), np.float32)
    LM = np.zeros((128, K), np.float32)
    V[blk, pos] = vals
    first = np.ones(len(nodes), bool)
    first[1:] = nodes[1:] != nodes[:-1]
    RM[blk, pos] = (~first).astype(np.float32)
    last = np.ones(len(nodes), bool)
    last[:-1] = nodes[1:] != nodes[:-1]
    LM[blk, pos] = last.astype(np.float32)
    return V, RM, LM


_CACHE = {}


def _get(key, builder, *a):
    if key not in _CACHE:
        _CACHE[key] = builder(*a)
    return _CACHE[key]


def kernel(x, edge_index, edge_feature, batch, _trace=False):
    ei = np.asarray(edge_index).astype(np.int64)
    p = np.asarray(edge_feature).astype(np.float32)[:, 0]
    batch = np.asarray(batch).astype(np.int64)
    uu = ei[0]
    tt = ei[1]

    # log list: every edge scatters at its target
    # s list: every edge at its target + non-self-loop edges at their source
    nsl = uu != tt
    nodes2 = np.concatenate([tt, uu[nsl]])
    vals2 = np.concatenate([p, p[nsl]])

    k1, k2 = K1, K2
    while True:
        p1 = _pack_list(tt, p, k1)
        if p1 is not None:
            break
        k1 += 32
    while True:
        p2 = _pack_list(nodes2, vals2, k2)
        if p2 is not None:
            break
        k2 += 32

    nc = _get((k1, k2), _build, k1, k2)

    misc = np.zeros((128, 1), np.float32)
    misc[0, 0] = float(batch.max())
    dina = np.concatenate([p1[0], misc], axis=1)
    dinm = np.concatenate([p1[1], p1[2]], axis=1).astype(np.float16)
    dinb = np.concatenate([p2[0], p2[1], p2[2]], axis=1).astype(np.float16)

    r = bass_utils.run_bass_kernel_spmd(
        nc, [{"dina": dina, "dinm": dinm, "dinb": dinb}], core_ids=[0],
        trace=_trace,
    )
    out = np.asarray(r.results[0]["out"], dtype=np.float32).reshape(1, 1)
    if _trace:
        kernel.last_results = (r,)
    return out


# revision 15
# speedup vs baseline: 1.0583x; 1.0583x over previous
"""Trainium2 Bass kernel for nn_ErdosLoss (graph loss function).

Math (reference reformulated, validated to ~1e-6 rel err):
  penalty:  log_score = scatter_add(log(1 - p + 1e-6), tgt)   over N nodes
            loss2 = mean(exp(log_score)) * 9600
  loss3:    p @ triu(H H^T, 1) @ p^T  ==  (||s||^2 - sum_e d_e p_e^2) / 2
            where s = scatter_add(p, tgt) + scatter_add(p, src | src != tgt),
            d_e = 2 - m_e, m_e = (src_e == tgt_e).
  out = loss2 + 200 * loss3 / num_graphs,  num_graphs = max(batch) + 1.

Single-launch single-core design (per-NEFF fixed overhead here is ~11us, so
any second launch loses):
  Host (index-only preprocessing; values are only reordered, never
  combined): sort the (node, value) scatter pairs by node, bin them so that
  partition p holds exactly nodes [32p, 32(p+1)), and emit aligned [128, K]
  arrays per list: V (values), RM (0 at each node's first pair, else 1),
  LM (1 at each node's last pair, else 0).

  Device: both lists ride ONE segment scan.  A single f16 tile BIG holds
  [Ln-region(K1) | V2(K2) | RM(K1+K2) | LM(K1+K2)]: ACT's Ln writes the
  left K1 columns in place, one f16 DMA fills the rest, so the combined
  tensor_tensor_scan (state = RM*state + v, resetting at segment starts)
  covers the log list and the s list back to back (each partition's s
  section starts with rm=0, so no cross-contamination).  d = r*LM isolates
  node totals; exp over the log half row-accumulates (masked slots count
  exp(0)=1 -> compile-time correction 128*K1-4000; empty nodes cancel);
  the s half yields sum(d^2) - sum(V2^2) via one accumulated product and
  an ACT Square.  R = [SC*er - SC*corr/128 | dsq]; ones-matmul partition
  reduce to PSUM [1,2]; res = F1*(100/num_graphs) + F0; one [1,1] DMA out.

  Latency layout: one explicit ACT table load (set 6 = Ln+Exp+Square+Copy,
  so no mid-kernel reloads); exactly two input transfers (V1+misc f32 on
  sync, the f16 block on scalar) since a queue's second transfer pays a
  ~0.7us re-arm gap; the exp mask-mult is split off first so ACT's Exp
  overlaps the s-half mask-mult.
"""

import numpy as np

import concourse.bacc as bacc
import concourse.mybir as mybir
import concourse.tile as tile
from concourse import bass_utils

F32 = mybir.dt.float32
F16 = mybir.dt.float16
ALU = mybir.AluOpType
ACT = mybir.ActivationFunctionType

N_NODES = 4000
PENALTY_SCALE = 16 * 200 * 3   # 9600
SC = PENALTY_SCALE / N_NODES   # 2.4
NPP = 32                       # nodes per partition (128 * 32 = 4096 >= 4000)

K1 = 64    # log-list slots per partition  (measured max 63)
K2 = 124   # s-list slots per partition    (measured max 120)


def _build(k1: int, k2: int):
    nc = bacc.Bacc("TRN2", target_bir_lowering=False, debug=False, num_devices=1)

    w = k1 + k2
    dv = nc.dram_tensor("dinv", [128, k1 + 1], F32, kind="ExternalInput").ap()
    df = nc.dram_tensor("dinf", [128, k2 + 2 * w], F16, kind="ExternalInput").ap()
    outd = nc.dram_tensor("out", [1, 1], F32, kind="ExternalOutput").ap()

    with tile.TileContext(nc) as tc:
        with (
            tc.tile_pool(name="pool", bufs=1) as pool,
            tc.tile_pool(name="psum", bufs=1, space="PSUM") as ppool,
        ):
            # one ACT table set covering Ln+Exp+Square+Copy (set 6); the
            # auto placement pass then inserts no further loads
            nc.scalar.add_instruction(mybir.InstLoadActFuncSet(
                name="actload6", ins=[], outs=[], act_func_set_id=6))

            # two input transfers, one per HWDGE queue (a second transfer on
            # the same queue pays a ~0.7us DGE re-arm gap)
            Bv = pool.tile([128, k1 + 1], F32, tag="Bv")
            nc.sync.dma_start(Bv[:], dv)
            BIG = pool.tile([128, k1 + k2 + 2 * w], F16, tag="BIG")
            nc.scalar.dma_start(BIG[:, k1:], df)

            # constants
            wb = pool.tile([128, 1], F32, tag="wb")
            nc.vector.memset(wb[:], 0.0)
            bias1 = pool.tile([128, 1], F32, tag="bias1")
            nc.vector.memset(bias1[:], 1.0 + 1e-6)
            ones_t = pool.tile([128, 1], F32, tag="ones_t")
            nc.vector.memset(ones_t[:], 1.0)
            # warm the DVE scan path on dummy data
            ws = pool.tile([128, 4], F32, tag="ws")
            nc.vector.tensor_tensor_scan(
                ws[:], wb[:].to_broadcast((128, 4)), wb[:].to_broadcast((128, 4)),
                0.0, op0=ALU.mult, op1=ALU.add,
            )

            V1 = Bv[:, 0:k1]
            bmax = Bv[0:1, k1:k1 + 1]
            VV = BIG[:, 0:w]
            V2 = BIG[:, k1:w]
            RM = BIG[:, w:2 * w]
            LM = BIG[:, 2 * w:3 * w]

            R = pool.tile([128, 2], F32, tag="R")

            # num_graphs early (only needs Bv): rng = 100 / (max(batch) + 1)
            ng1 = pool.tile([1, 1], F32, tag="ng1")
            nc.vector.tensor_scalar(ng1[:], bmax, 1.0, 0.01,
                                    op0=ALU.add, op1=ALU.mult)
            rng = pool.tile([1, 1], F32, tag="rng")
            nc.vector.reciprocal(rng[:], ng1[:])

            # ---- Ln writes BIG's left K1 columns, completing VV
            nc.scalar.activation(BIG[:, 0:k1], V1, ACT.Ln, scale=-1.0,
                                 bias=bias1[:])
            # sum V2^2 on ACT while DVE scans (accum feeds dsq)
            sq3 = pool.tile([128, k2], F16, tag="sq3")
            Rt2 = pool.tile([128, 1], F32, tag="Rt2")
            nc.scalar.activation(sq3[:], V2, ACT.Square, bias=wb[:],
                                 accum_out=Rt2[:])

            # ---- one combined segment scan over [log | s]
            r = pool.tile([128, w], F32, tag="r")
            nc.vector.tensor_tensor_scan(
                r[:], RM, VV, 0.0, op0=ALU.mult, op1=ALU.add
            )
            # exp half first so ACT's Exp overlaps the s-half mult
            dmc1 = pool.tile([128, k1], F32, tag="dmc1")
            nc.vector.tensor_tensor(dmc1[:], r[:, 0:k1], LM[:, 0:k1],
                                    op=ALU.mult)
            e1 = pool.tile([128, k1], F32, tag="e1")
            er = pool.tile([128, 1], F32, tag="er")
            nc.scalar.activation(e1[:], dmc1[:], ACT.Exp, bias=wb[:],
                                 accum_out=er[:])
            dmc2 = pool.tile([128, k2], F32, tag="dmc2")
            nc.vector.tensor_tensor(dmc2[:], r[:, k1:w], LM[:, k1:w],
                                    op=ALU.mult)
            sq2 = pool.tile([128, k2], F32, tag="sq2")
            Rt1 = pool.tile([128, 1], F32, tag="Rt1")
            nc.vector.scalar_tensor_tensor(
                sq2[:], dmc2[:], 1.0, dmc2[:],
                op0=ALU.mult, op1=ALU.mult, accum_out=Rt1[:],
            )
            nc.vector.tensor_tensor(R[:, 1:2], Rt1[:], Rt2[:], op=ALU.subtract)
            corr = float(128 * k1 - N_NODES)
            nc.vector.tensor_scalar(R[:, 0:1], er[:], SC, -corr * SC / 128.0,
                                    op0=ALU.mult, op1=ALU.add)

            # ---- partition reduce + final tail (DVE only)
            F = ppool.tile([1, 2], F32, tag="F")
            nc.tensor.matmul(F[:], ones_t[:], R[:], start=True, stop=True)
            cp0 = pool.tile([1, 1], F32, tag="cp0")
            nc.vector.tensor_copy(cp0[:], F[:, 0:1])
            res2 = pool.tile([1, 1], F32, tag="res2")
            nc.vector.scalar_tensor_tensor(
                res2[:], F[:, 1:2], rng[:], cp0[:],
                op0=ALU.mult, op1=ALU.add,
            )
            nc.sync.dma_start(outd, res2[:])

    nc.compile()
    return nc


def _pack_list(nodes, vals, K):
    """Sort (node, value) pairs, bin node n into partition n // 32, emit
    aligned V / RM / LM [128, K] arrays.  Index work + reordering only."""
    order = np.argsort(nodes, kind="stable")
    nodes = nodes[order]
    vals = vals[order]
    blk = nodes // NPP
    starts = np.searchsorted(blk, np.arange(128), "left")
    cnt = np.bincount(blk, minlength=128)
    if cnt.max() > K:
        return None
    pos = np.arange(len(nodes)) - starts[blk]

    V = np.zeros((128, K), np.float32)
    RM = np.ones((128, K), np.float32)
    LM = np.zeros((128, K), np.float32)
    V[blk, pos] = vals
    first = np.ones(len(nodes), bool)
    first[1:] = nodes[1:] != nodes[:-1]
    RM[blk, pos] = (~first).astype(np.float32)
    last = np.ones(len(nodes), bool)
    last[:-1] = nodes[1:] != nodes[:-1]
    LM[blk, pos] = last.astype(np.float32)
    return V, RM, LM


_CACHE = {}


def _get(key, builder, *a):
    if key not in _CACHE:
        _CACHE[key] = builder(*a)
    return _CACHE[key]


def kernel(x, edge_index, edge_feature, batch, _trace=False):
    ei = np.asarray(edge_index).astype(np.int64)
    p = np.asarray(edge_feature).astype(np.float32)[:, 0]
    batch = np.asarray(batch).astype(np.int64)
    uu = ei[0]
    tt = ei[1]

    # log list: every edge scatters at its target
    # s list: every edge at its target + non-self-loop edges at their source
    nsl = uu != tt
    nodes2 = np.concatenate([tt, uu[nsl]])
    vals2 = np.concatenate([p, p[nsl]])

    k1, k2 = K1, K2
    while True:
        p1 = _pack_list(tt, p, k1)
        if p1 is not None:
            break
        k1 += 32
    while True:
        p2 = _pack_list(nodes2, vals2, k2)
        if p2 is not None:
            break
        k2 += 32

    nc = _get((k1, k2), _build, k1, k2)

    misc = np.zeros((128, 1), np.float32)
    misc[0, 0] = float(batch.max())
    dinv = np.concatenate([p1[0], misc], axis=1)
    dinf = np.concatenate(
        [p2[0], p1[1], p2[1], p1[2], p2[2]], axis=1
    ).astype(np.float16)   # V2 | RM1 | RM2 | LM1 | LM2

    r = bass_utils.run_bass_kernel_spmd(
        nc, [{"dinv": dinv, "dinf": dinf}], core_ids=[0], trace=_trace,
    )
    out = np.asarray(r.results[0]["out"], dtype=np.float32).reshape(1, 1)
    if _trace:
        kernel.last_results = (r,)
    return out
